# revision 33
# baseline (speedup 1.0000x reference)
"""Trainium2 Bass kernel for nn_Block_23338852286694 (dense transformer block).

Sharding: 8 cores = 4 batches x 2 query-halves. Each core computes the full
block for its 512 query tokens (K/V work over the full 1024-token sequence is
duplicated across the pair of cores sharing a batch; no collectives).

On-chip dataflow is feature-major (activations stored transposed, [E, T]).
All weights are host-prepacked into [128, 6, 768] slices so every weight load
is a single 128-descriptor DMA (18KB/partition contiguous). Attention outputs
stay in SBUF via partition-offset engine writes (no DRAM staging); softmax
denominators are batched per attention pass and broadcast with a
selection-matrix matmul (no DMA bounces). All matmuls run in float32r.
"""
import numpy as np

import concourse.bass as bass
import concourse.bacc as bacc
import concourse.mybir as mybir
import concourse.tile as tile
from concourse.bass_utils import run_bass_kernel_spmd

F32 = mybir.dt.float32
F16 = mybir.dt.float16
F32R = mybir.dt.float32r
AF = mybir.ActivationFunctionType
ALU = mybir.AluOpType

B, S, SE = 4, 1024, 1024
E, H, M, D = 768, 12, 100, 64
KC = E // 128             # 6 feature chunks
Q = S // 2                # 512 query tokens per core
EPS = 1e-5
NKT = S // 128            # 8 key tiles
RSQ2 = float(1.0 / np.sqrt(2.0))

# packed per-partition bias column offsets
BC_Q, BC_K, BC_PROJ, BC_MA, BC_FCQ, BC_FCK, BC_EP, BC_A1, BC_A2, BC_FC, BC_PJ = (
    0, 6, 12, 18, 24, 30, 36, 42, 48, 54, 78)
NB = 84

# weight slice order in the packed wpk tensor [NW, 128, KC, 768]
(W_QKQ, W_QKK, W_VS, W_PROJ, W_MA0, W_MA1, W_Q, W_K, W_V, W_EP,
 W_A10, W_A11, W_A20, W_A21, W_FC0, W_FC1, W_FC2, W_FC3,
 W_PJ00, W_PJ01, W_PJ10, W_PJ11) = range(22)
NW = 22


def _row_bcast_dram(ap, parts):
    """DRAM row [N] -> AP readable as [parts, N] (partition-broadcast)."""
    return bass.AP(tensor=ap.tensor, offset=ap.offset,
                   ap=[[0, parts], list(ap.ap[-1])])


def build_program():
    nc = bacc.Bacc(trn_type="TRN2")

    x_pct = nc.dram_tensor("x_pct", [128, KC, S], F32R, kind="ExternalInput")
    xq_pct = nc.dram_tensor("xq_pct", [128, KC, Q], F32R, kind="ExternalInput")
    enc_pct = nc.dram_tensor("enc_pct", [2, 128, KC, SE], F32R,
                             kind="ExternalInput")
    wpk = nc.dram_tensor("wpk", [NW, 128, KC, 768], F32R, kind="ExternalInput")
    maskmul = nc.dram_tensor("maskmul", [128, NKT], F32, kind="ExternalInput")
    mkT = nc.dram_tensor("mkT", [128, KC, M], F32R, kind="ExternalInput")
    mvA = nc.dram_tensor("mvA", [M, H * 65], F32R, kind="ExternalInput")
    bcols = nc.dram_tensor("bcols", [128, NB], F32, kind="ExternalInput")
    bv_self = nc.dram_tensor("bv_self", [E], F16, kind="ExternalInput")
    bv_enc = nc.dram_tensor("bv_enc", [E], F16, kind="ExternalInput")
    outP = nc.dram_tensor("outP", [128, KC, Q], F32, kind="ExternalOutput")

    with tile.TileContext(nc) as tc:
        with nc.allow_low_precision(reason="f32r tiles store full fp32 bits"):
            _emit(nc, tc, x_pct, xq_pct, enc_pct, wpk, maskmul, mkT, mvA,
                  bcols, bv_self, bv_enc, outP)
    nc.compile()
    return nc


def _emit(nc, tc, x_pct, xq_pct, enc_pct, wpk, maskmul, mkT, mvA, bcols,
          bv_self, bv_enc, outP):
    from contextlib import ExitStack
    ctx = ExitStack()
    with ctx:
        consts = ctx.enter_context(tc.tile_pool(name="consts", bufs=1))
        wp = ctx.enter_context(tc.tile_pool(name="wp", bufs=2))
        big24 = ctx.enter_context(tc.tile_pool(name="big24", bufs=1))
        a24 = ctx.enter_context(tc.tile_pool(name="a24", bufs=1))
        q12 = ctx.enter_context(tc.tile_pool(name="q12", bufs=1))
        h12 = ctx.enter_context(tc.tile_pool(name="h12", bufs=1))
        r12 = ctx.enter_context(tc.tile_pool(name="r12", bufs=1))
        m12 = ctx.enter_context(tc.tile_pool(name="m12", bufs=1))
        sm12 = ctx.enter_context(tc.tile_pool(name="sm12", bufs=1))
        vsbp = ctx.enter_context(tc.tile_pool(name="vsbp", bufs=1))
        kchp = ctx.enter_context(tc.tile_pool(name="kchp", bufs=2))
        bcp = ctx.enter_context(tc.tile_pool(name="bcp", bufs=2))
        tmp = ctx.enter_context(tc.tile_pool(name="tmp", bufs=2))
        ptp = ctx.enter_context(tc.tile_pool(name="ptp", bufs=2))
        rcpp = ctx.enter_context(tc.tile_pool(name="rcpp", bufs=2))
        plin = ctx.enter_context(tc.tile_pool(name="plin", bufs=2, space="PSUM"))
        psc = ctx.enter_context(tc.tile_pool(name="psc", bufs=2, space="PSUM"))
        pav = ctx.enter_context(tc.tile_pool(name="pav", bufs=2, space="PSUM"))

        # ---- constants ----
        ones_mm = consts.tile([128, 1], F32R)
        onesf = consts.tile([128, 12], F32)
        nc.vector.memset(onesf, 1.0)
        nc.vector.tensor_copy(out=ones_mm, in_=onesf[:, 0:1])
        ones_f32 = consts.tile([128, 128], F32)
        nc.vector.memset(ones_f32, 1.0)
        ones_bc = consts.tile([128, 128], F32R)  # broadcast rows of 1.0
        nc.vector.tensor_copy(out=ones_bc, in_=ones_f32)
        oneE_bc = consts.tile([128, 128], F32R)  # broadcast rows of 1/E
        nc.vector.tensor_scalar(out=oneE_bc, in0=ones_f32,
                                scalar1=float(1.0 / E), scalar2=None,
                                op0=ALU.mult)
        bc = consts.tile([128, NB], F32)
        nc.sync.dma_start(out=bc, in_=bcols[:, :])
        mm_sb = consts.tile([128, NKT], F32)
        nc.sync.dma_start(out=mm_sb, in_=maskmul[:, :])
        mk_sb = consts.tile([128, KC, M], F32R)
        nc.sync.dma_start(out=mk_sb, in_=mkT[:, :, :])
        mv_sb = consts.tile([M, H * 65], F32R)
        nc.sync.dma_start(out=mv_sb, in_=mvA[:, :])
        bvb_s = consts.tile([128, E], F16)
        nc.sync.dma_start(out=bvb_s, in_=_row_bcast_dram(bv_self[:], 128))
        bvb_e = consts.tile([128, E], F16)
        nc.sync.dma_start(out=bvb_e, in_=_row_bcast_dram(bv_enc[:], 128))
        eps_t = consts.tile([128, 1], F32)
        nc.vector.memset(eps_t, EPS)

        def load_w(idx, name="wt"):
            t = wp.tile([128, KC, 768], F32R, tag="w18", name=name)
            nc.sync.dma_start(out=t, in_=wpk[idx, :, :, :])
            return t

        def layernorm(chunks, T, out_chunks=None):
            """Feature-major layernorm. chunks: KC APs [128, T] f32r.
            out_chunks None -> in place."""
            nt = T // 512
            inplace = out_chunks is None
            srows = []
            for t in range(nt):
                ps_s = plin.tile([128, 512], F32, tag="lin", name="ps_s")
                ps_q = plin.tile([128, 512], F32, tag="lin", name="ps_q")
                sl = slice(t * 512, (t + 1) * 512)
                for c in range(KC):
                    src = chunks[c][:, sl]
                    sq = tmp.tile([128, 512], F32R, tag="t2", name="sq")
                    eng = nc.gpsimd if c % 2 else nc.vector
                    eng.tensor_tensor(out=sq, in0=src.bitcast(F32),
                                      in1=src.bitcast(F32), op=ALU.mult)
                    nc.tensor.matmul(ps_s[0:1, :], ones_mm, src,
                                     start=(c == 0), stop=(c == KC - 1))
                    nc.tensor.matmul(ps_q[0:1, :], ones_mm, sq,
                                     start=(c == 0), stop=(c == KC - 1))
                sr = tmp.tile([128, 512], F32R, tag="t2", name="sr")
                nc.scalar.activation(out=sr[0:1, :], in_=ps_s[0:1, :],
                                     func=AF.Identity, scale=1.0)
                nc.scalar.activation(out=sr[32:33, :], in_=ps_q[0:1, :],
                                     func=AF.Identity, scale=1.0)
                srows.append(sr)
            mu_b = psc.tile([128, 1024], F32, tag="sc", name="mub")
            e2_b = psc.tile([128, 1024], F32, tag="sc", name="e2b")
            for t in range(nt):
                sl = slice(t * 512, (t + 1) * 512)
                nc.tensor.matmul(mu_b[:, sl], oneE_bc[0:1, :],
                                 srows[t][0:1, :], start=True, stop=True)
                nc.tensor.matmul(e2_b[:, sl], oneE_bc[32:33, :],
                                 srows[t][32:33, :], start=True, stop=True)
            rs_sb = bcp.tile([128, 1024], F16, tag="bc8", name="rs")
            murs_sb = bcp.tile([128, 1024], F16, tag="bc8", name="murs")
            for t in range(nt):
                sl = slice(t * 512, (t + 1) * 512)
                mu_s = tmp.tile([128, 512], F32, tag="t2", name="mu_s")
                nc.vector.tensor_copy(out=mu_s, in_=mu_b[:, sl])
                msq = tmp.tile([128, 512], F32, tag="t2", name="msq")
                nc.vector.tensor_tensor(out=msq, in0=mu_s, in1=mu_s,
                                        op=ALU.mult)
                nc.vector.tensor_tensor(out=e2_b[:, sl], in0=e2_b[:, sl],
                                        in1=msq, op=ALU.subtract)
                sd = tmp.tile([128, 512], F32, tag="t2", name="sd")
                nc.scalar.activation(out=sd, in_=e2_b[:, sl], func=AF.Sqrt,
                                     bias=eps_t[:, :], scale=1.0)
                nc.vector.reciprocal(out=rs_sb[:, sl], in_=sd)
                nc.vector.tensor_tensor(out=murs_sb[:, sl], in0=mu_s,
                                        in1=rs_sb[:, sl], op=ALU.mult)
            for c in range(KC):
                for t in range(nt):
                    sl = slice(t * 512, (t + 1) * 512)
                    src = chunks[c][:, sl]
                    o = src if inplace else out_chunks[c][:, sl]
                    eng = nc.gpsimd if c % 2 else nc.vector
                    eng.tensor_tensor(out=o, in0=src.bitcast(F32),
                                      in1=rs_sb[:, sl], op=ALU.mult)
                    nc.vector.tensor_tensor(out=o, in0=o.bitcast(F32),
                                            in1=murs_sb[:, sl],
                                            op=ALU.subtract)

        def lin_psum(ft):
            """Alternate linear accumulators across both PSUM pools for a
            4-deep drain pipeline (only outside attention phases)."""
            if ft % 2:
                return psc.tile([128, 1024], F32, tag="sc", name="lps")[:, 0:512]
            return plin.tile([128, 512], F32, tag="lin", name="lp")[:, :]

        def linear_ft(ft, terms, bias_col, act, out_ap, alt=True):
            """terms: list of (wtile, kc_in_tile, rhs [128, Q] AP)."""
            ps = lin_psum(ft) if alt else plin.tile(
                [128, 512], F32, tag="lin", name="lp")[:, :]
            n = len(terms)
            for i, (wt, kci, rhs) in enumerate(terms):
                nc.tensor.matmul(ps, wt[:, kci, ft * 128:(ft + 1) * 128],
                                 rhs, start=(i == 0), stop=(i == n - 1))
            nc.scalar.activation(
                out=out_ap, in_=ps, func=act,
                bias=bc[:, bias_col + ft:bias_col + ft + 1], scale=1.0)

        def make_v(v_tile, src_fn, wv, bias_b, masked):
            """V production into v_tile [128, NKT, H*65] (token-major)."""
            for tt in range(NKT):
                for h0 in (0, 6):
                    c0 = h0 * 64
                    ps = plin.tile([128, 512], F32, tag="lin", name="vps")
                    for kc in range(KC):
                        nc.tensor.matmul(
                            ps[:, 0:384],
                            src_fn(kc)[:, tt * 128:(tt + 1) * 128],
                            wv[:, kc, c0:c0 + 384],
                            start=(kc == 0), stop=(kc == KC - 1))
                    vrow = v_tile[:, tt, :].rearrange("p (h c) -> p h c", c=65)
                    nc.vector.tensor_tensor(
                        out=vrow[:, h0:h0 + 6, 0:64],
                        in0=ps[:, 0:384].rearrange("p (h c) -> p h c", c=64),
                        in1=bias_b[:, c0:c0 + 384].rearrange(
                            "p (h c) -> p h c", c=64),
                        op=ALU.add)
                if masked:
                    nc.vector.tensor_scalar(
                        out=v_tile[:, tt, :],
                        in0=v_tile[:, tt, :].bitcast(F32),
                        scalar1=mm_sb[:, tt:tt + 1], scalar2=None,
                        op0=ALU.mult)

        def init_ones_cols(v_tile):
            for tt in range(NKT):
                vrow = v_tile[:, tt, :].rearrange("p (h c) -> p h c", c=65)
                nc.vector.tensor_copy(
                    out=vrow[:, :, 64:65],
                    in_=onesf[:, :].rearrange("p (h o) -> p h o", o=1))

        def attention(kch_fn, v_tile, q_tile, out24, scale, with_mem):
            """q_tile [128, KC, Q]; raw (unnormalized) head outputs into
            out24[:, c, :] rows off..off+64 (cols 6+c for memory); per-chunk
            raw denominators staged at rows 0/32 of a small tile, broadcast
            with K=1 ones-matmuls, then one reciprocal+multiply per chunk."""
            for c in range(KC):
                kch = kch_fn(c)
                den2 = rcpp.tile([33, 512], F32R, tag="den", name="den2")
                den2m = None
                if with_mem:
                    den2m = rcpp.tile([33, 512], F32R, tag="den", name="den2m")
                for hh in range(2):
                    h = 2 * c + hh
                    off = hh * 64
                    dr = 32 * hh
                    av = pav.tile([65, 512], F32, tag="av", name="av")
                    for half in range(NKT // 2):
                        sc = psc.tile([128, 1024], F32, tag="sc", name="sc")
                        for j in range(2):
                            kt = half * 2 + j
                            nc.tensor.matmul(
                                sc[:, j * 512:(j + 1) * 512],
                                kch[off:off + 64, kt * 128:(kt + 1) * 128],
                                q_tile[off:off + 64, c, :],
                                start=True, stop=True)
                        pt = ptp.tile([128, 1024], F32R, tag="pt", name="pt")
                        nc.scalar.activation(out=pt, in_=sc[:, :], func=AF.Exp,
                                             scale=scale)
                        for j in range(2):
                            kt = half * 2 + j
                            nc.tensor.matmul(
                                av[:, :], v_tile[:, kt, h * 65:(h + 1) * 65],
                                pt[:, j * 512:(j + 1) * 512],
                                start=(kt == 0), stop=(kt == NKT - 1))
                    nc.vector.tensor_copy(out=den2[dr:dr + 1, :],
                                          in_=av[64:65, :])
                    if hh == 0:
                        nc.scalar.activation(out=out24[off:off + 64, c, :],
                                             in_=av[0:64, :], func=AF.Identity,
                                             scale=1.0)
                    else:
                        nc.vector.tensor_copy(out=out24[off:off + 64, c, :],
                                              in_=av[0:64, :])
                    if with_mem:
                        scm = plin.tile([128, 512], F32, tag="lin", name="scm")
                        nc.tensor.matmul(scm[0:M, 0:512],
                                         mk_sb[off:off + 64, c, :],
                                         q_tile[off:off + 64, c, :],
                                         start=True, stop=True)
                        pmt = ptp.tile([128, 1024], F32R, tag="pt", name="pmt")
                        nc.scalar.activation(out=pmt[0:M, 0:512],
                                             in_=scm[0:M, :],
                                             func=AF.Exp, scale=1.0)
                        av1 = pav.tile([65, 512], F32, tag="av", name="av1")
                        nc.tensor.matmul(av1[:, :],
                                         mv_sb[:, h * 65:(h + 1) * 65],
                                         pmt[0:M, 0:512], start=True,
                                         stop=True)
                        nc.scalar.activation(out=den2m[dr:dr + 1, :],
                                             in_=av1[64:65, :],
                                             func=AF.Identity, scale=1.0)
                        if hh == 0:
                            nc.scalar.activation(
                                out=out24[off:off + 64, KC + c, :],
                                in_=av1[0:64, :], func=AF.Identity, scale=1.0)
                        else:
                            nc.vector.tensor_copy(
                                out=out24[off:off + 64, KC + c, :],
                                in_=av1[0:64, :])
                blocks = [(0, den2)] + ([(KC, den2m)] if with_mem else [])
                for blk, dt in blocks:
                    bpa = plin.tile([128, 512], F32, tag="lin", name="bpa")
                    bpb = plin.tile([128, 512], F32, tag="lin", name="bpb")
                    nc.tensor.matmul(bpa[0:64, :], ones_bc[0:1, 0:64],
                                     dt[0:1, :], start=True, stop=True)
                    nc.tensor.matmul(bpb[0:64, :], ones_bc[32:33, 0:64],
                                     dt[32:33, :], start=True, stop=True)
                    nc.vector.reciprocal(out=bpa[0:64, :], in_=bpa[0:64, :])
                    nc.vector.reciprocal(out=bpb[0:64, :], in_=bpb[0:64, :])
                    nc.vector.tensor_tensor(
                        out=out24[0:64, blk + c, :],
                        in0=out24[0:64, blk + c, :].bitcast(F32),
                        in1=bpa[0:64, :], op=ALU.mult)
                    nc.vector.tensor_tensor(
                        out=out24[64:128, blk + c, :],
                        in0=out24[64:128, blk + c, :].bitcast(F32),
                        in1=bpb[0:64, :], op=ALU.mult)

        # ---- persistent tiles ----
        v_sb = vsbp.tile([128, NKT, H * 65], F32R, tag="vsb")
        init_ones_cols(v_sb)

        # ======== phase A: load x, LN in place ========
        xh = big24.tile([128, KC, S], F32R, tag="big24", name="xh")
        for c in range(KC):
            nc.sync.dma_start(out=xh[:, c, :], in_=x_pct[:, c, :])
        xqh = h12.tile([128, KC, Q], F32R, tag="h12", name="xqh")
        nc.sync.dma_start(out=xqh, in_=xq_pct[:, :, :])
        layernorm([xh[:, c, :] for c in range(KC)], S)
        layernorm([xqh[:, c, :] for c in range(KC)], Q)

        # ======== phase B: v, q, k, self+memory attention ========
        w_vs = load_w(W_VS, "wvs")
        make_v(v_sb, lambda kc: xh[:, kc, :], w_vs, bvb_s, masked=False)
        w_qq = load_w(W_QKQ, "wqq")
        qT = q12.tile([128, KC, Q], F32R, tag="q12", name="qT")
        for ft in range(KC):
            linear_ft(ft, [(w_qq, kc, xqh[:, kc, :]) for kc in range(KC)],
                      BC_Q, AF.Identity, qT[:, ft, :])
        w_qk = load_w(W_QKK, "wqk")

        def self_kchunk(c):
            kt = kchp.tile([128, S], F32R, tag="kch", name="kchs")
            for t in range(2):
                sl = slice(t * 512, (t + 1) * 512)
                ps = plin.tile([128, 512], F32, tag="lin", name="kps")
                for kc in range(KC):
                    nc.tensor.matmul(ps[:, :],
                                     w_qk[:, kc, c * 128:(c + 1) * 128],
                                     xh[:, kc, sl], start=(kc == 0),
                                     stop=(kc == KC - 1))
                nc.scalar.activation(out=kt[:, sl], in_=ps[:, :],
                                     func=AF.Identity,
                                     bias=bc[:, BC_K + c:BC_K + c + 1],
                                     scale=1.0)
            return kt

        gAB = a24.tile([128, 12, Q], F32R, tag="a24", name="gAB")
        attention(self_kchunk, v_sb, qT, gAB, 1.0, with_mem=True)

        # ======== phase B4: memory gate + attn_proj + residual ========
        w_ma0 = load_w(W_MA0, "wma0")
        w_ma1 = load_w(W_MA1, "wma1")
        aN = q12.tile([128, KC, Q], F32R, tag="q12", name="aN")
        for ft in range(KC):
            ps = lin_psum(ft)
            for i in range(12):
                wt = w_ma0 if i < KC else w_ma1
                nc.tensor.matmul(ps,
                                 wt[:, i % KC, ft * 128:(ft + 1) * 128],
                                 gAB[:, i, :], start=(i == 0), stop=(i == 11))
            al = tmp.tile([128, 512], F32, tag="t2", name="al")
            nc.scalar.activation(out=al, in_=ps, func=AF.Sigmoid,
                                 bias=bc[:, BC_MA + ft:BC_MA + ft + 1],
                                 scale=1.0)
            d = tmp.tile([128, 512], F32, tag="t2", name="d")
            nc.vector.tensor_tensor(out=d, in0=gAB[:, ft, :].bitcast(F32),
                                    in1=gAB[:, KC + ft, :].bitcast(F32),
                                    op=ALU.subtract)
            eng = nc.gpsimd if ft % 2 else nc.vector
            eng.tensor_tensor(out=d, in0=al, in1=d, op=ALU.mult)
            nc.vector.tensor_tensor(out=aN[:, ft, :],
                                    in0=gAB[:, KC + ft, :].bitcast(F32),
                                    in1=d, op=ALU.add)
        w_proj = load_w(W_PROJ, "wproj")
        xq_raw = m12.tile([128, KC, Q], F32R, tag="m12", name="xq_raw")
        nc.sync.dma_start(out=xq_raw, in_=xq_pct[:, :, :])
        aT = sm12.tile([128, KC, Q], F32R, tag="sm12", name="aT")
        for ft in range(KC):
            ps = lin_psum(ft)
            for kc in range(KC):
                nc.tensor.matmul(ps,
                                 w_proj[:, kc, ft * 128:(ft + 1) * 128],
                                 aN[:, kc, :], start=(kc == 0),
                                 stop=(kc == KC - 1))
            nc.vector.scalar_tensor_tensor(
                out=aT[:, ft, :], in0=ps,
                scalar=bc[:, BC_PROJ + ft:BC_PROJ + ft + 1],
                in1=xq_raw[:, ft, :].bitcast(F32), op0=ALU.add, op1=ALU.add)

        # ======== encoder 0 load + LN (overlaps B4/C) ========
        eh0 = big24.tile([128, KC, SE], F32R, tag="big24", name="eh0")
        for c in range(KC):
            nc.sync.dma_start(out=eh0[:, c, :], in_=enc_pct[0, :, c, :])
        layernorm([eh0[:, c, :] for c in range(KC)], SE)

        # ======== phase C: hahat + qe ========
        hah = h12.tile([128, KC, Q], F32R, tag="h12", name="hah")
        layernorm([aT[:, c, :] for c in range(KC)], Q,
                  [hah[:, c, :] for c in range(KC)])
        w_fcq = load_w(W_Q, "wfcq")
        qeT = q12.tile([128, KC, Q], F32R, tag="q12", name="qeT")
        for ft in range(KC):
            linear_ft(ft, [(w_fcq, kc, hah[:, kc, :]) for kc in range(KC)],
                      BC_FCQ, AF.Identity, qeT[:, ft, :])

        # ======== phase D: two cross-attentions ========
        e1p = r12.tile([128, KC, Q], F32, tag="r12", name="e1p")
        asum = m12.tile([128, KC, Q], F32R, tag="m12", name="asum")

        def enc_attention(eh, out24):
            w_fcv = load_w(W_V, "wfcv")
            make_v(v_sb, lambda kc: eh[:, kc, :], w_fcv, bvb_e, masked=True)
            w_fck = load_w(W_K, "wfck")

            def enc_kchunk(c):
                kt = kchp.tile([128, S], F32R, tag="kch", name="kche")
                for t in range(2):
                    sl = slice(t * 512, (t + 1) * 512)
                    ps = plin.tile([128, 512], F32, tag="lin", name="keps")
                    for kc in range(KC):
                        nc.tensor.matmul(ps[:, :],
                                         w_fck[:, kc, c * 128:(c + 1) * 128],
                                         eh[:, kc, sl], start=(kc == 0),
                                         stop=(kc == KC - 1))
                    nc.scalar.activation(out=kt[:, sl], in_=ps[:, :],
                                         func=AF.Identity,
                                         bias=bc[:, BC_FCK + c:BC_FCK + c + 1],
                                         scale=1.0)
                return kt

            attention(enc_kchunk, v_sb, qeT, out24, 0.125, with_mem=False)

        def enc_proj(ee):
            w_ep = load_w(W_EP, "wep")
            for ft in range(KC):
                linear_ft(ft, [(w_ep, kc, ee[:, kc, :]) for kc in range(KC)],
                          BC_EP, AF.Identity, ee[:, KC + ft, :])

        def enc_gates(e, ee):
            wa0 = load_w(W_A10 if e == 0 else W_A20, "wa0")
            wa1 = load_w(W_A11 if e == 0 else W_A21, "wa1")
            bcol0 = BC_A1 if e == 0 else BC_A2
            for ft in range(KC):
                ps = lin_psum(ft)
                for i in range(12):
                    wt = wa0 if i < KC else wa1
                    rhs = aT[:, i, :] if i < KC else ee[:, i, :]
                    nc.tensor.matmul(ps,
                                     wt[:, i % KC, ft * 128:(ft + 1) * 128],
                                     rhs, start=(i == 0), stop=(i == 11))
                al = tmp.tile([128, 512], F32, tag="t2", name="alE")
                nc.scalar.activation(out=al, in_=ps, func=AF.Sigmoid,
                                     bias=bc[:, bcol0 + ft:bcol0 + ft + 1],
                                     scale=1.0)
                d = tmp.tile([128, 512], F32, tag="t2", name="dE")
                nc.vector.tensor_tensor(out=d, in0=aT[:, ft, :].bitcast(F32),
                                        in1=ee[:, KC + ft, :].bitcast(F32),
                                        op=ALU.subtract)
                eng = nc.gpsimd if ft % 2 else nc.vector
                eng.tensor_tensor(out=d, in0=al, in1=d, op=ALU.mult)
                if e == 0:
                    nc.vector.tensor_tensor(out=e1p[:, ft, :],
                                            in0=ee[:, KC + ft, :].bitcast(F32),
                                            in1=d, op=ALU.add)
                else:
                    nc.vector.tensor_tensor(out=d,
                                            in0=ee[:, KC + ft, :].bitcast(F32),
                                            in1=d, op=ALU.add)
                    nc.vector.tensor_tensor(out=asum[:, ft, :],
                                            in0=e1p[:, ft, :], in1=d,
                                            op=ALU.add)

        ee0 = a24.tile([128, 12, Q], F32R, tag="a24", name="ee0")
        enc_attention(eh0, ee0)
        enc_proj(ee0)
        # encoder 1 load + LN (overlaps e0 ep/gates)
        eh1 = big24.tile([128, KC, SE], F32R, tag="big24", name="eh1")
        for c in range(KC):
            nc.sync.dma_start(out=eh1[:, c, :], in_=enc_pct[1, :, c, :])
        layernorm([eh1[:, c, :] for c in range(KC)], SE)
        enc_gates(0, ee0)
        ee1 = a24.tile([128, 12, Q], F32R, tag="a24", name="ee1")
        enc_attention(eh1, ee1)
        enc_proj(ee1)
        enc_gates(1, ee1)

        # ======== phase E: MLP + final residual ========
        hm2 = h12.tile([128, KC, Q], F32R, tag="h12", name="hm2")
        layernorm([asum[:, c, :] for c in range(KC)], Q,
                  [hm2[:, c, :] for c in range(KC)])
        mstage = sm12.tile([128, KC, Q], F32, tag="sm12", name="mstage")
        for mh in range(2):
            wfa = load_w(W_FC0 + 2 * mh, "wfa")
            wfb = load_w(W_FC1 + 2 * mh, "wfb")
            mpool = big24 if mh == 0 else a24
            mT = mpool.tile([128, 12, Q], F32R,
                            tag="big24" if mh == 0 else "a24", name="mT")
            for ft in range(12):
                wt = wfa if ft < KC else wfb
                linear_ft(ft % KC,
                          [(wt, kc, hm2[:, kc, :]) for kc in range(KC)],
                          BC_FC + 12 * mh + (0 if ft < KC else KC),
                          AF.Gelu_apprx_tanh, mT[:, ft, :])
            wpa = load_w(W_PJ00 + 2 * mh, "wpa")
            wpb = load_w(W_PJ01 + 2 * mh, "wpb")
            for ft in range(KC):
                ps = lin_psum(ft)
                for i in range(12):
                    wt = wpa if i < KC else wpb
                    nc.tensor.matmul(ps,
                                     wt[:, i % KC, ft * 128:(ft + 1) * 128],
                                     mT[:, i, :], start=(i == 0),
                                     stop=(i == 11))
                if mh == 0:
                    nc.scalar.activation(out=mstage[:, ft, :], in_=ps,
                                         func=AF.Identity,
                                         bias=bc[:, BC_PJ + ft:BC_PJ + ft + 1],
                                         scale=1.0)
                else:
                    t = tmp.tile([128, 512], F32, tag="t2", name="mo")
                    nc.vector.scalar_tensor_tensor(
                        out=t, in0=asum[:, ft, :].bitcast(F32), scalar=RSQ2,
                        in1=ps, op0=ALU.mult, op1=ALU.add)
                    ot = tmp.tile([128, 512], F32, tag="t2", name="ot")
                    nc.vector.tensor_tensor(out=ot, in0=t,
                                            in1=mstage[:, ft, :], op=ALU.add)
                    nc.sync.dma_start(out=outP[:, ft, :], in_=ot)


_NC_CACHE = None


def _get_nc():
    global _NC_CACHE
    if _NC_CACHE is None:
        _NC_CACHE = build_program()
    return _NC_CACHE


def _pack_bias_cols(seg_biases):
    bcols = np.zeros((128, NB), np.float32)
    for col0, b in seg_biases:
        nf = b.shape[0] // 128
        bcols[:, col0:col0 + nf] = b.reshape(nf, 128).T
    return bcols


def _pack_w(blk):
    """[768, F] -> [128, 6, F] per-partition contiguous."""
    return np.ascontiguousarray(
        blk.reshape(KC, 128, blk.shape[1]).transpose(1, 0, 2))


def kernel(x, encoder_features, mask_encoder, ln1_g, ln1_b, ln2_g, ln2_b,
           c_attn_w, c_attn_b, attn_proj_w, attn_proj_b,
           memory_features, mem_attn_w, mem_attn_b, mem_alpha_w, mem_alpha_b,
           fcq_w, fcq_b, fck_w, fck_b, fcv_w, fcv_b, enc_proj_w, enc_proj_b,
           fc_alpha1_w, fc_alpha1_b, fc_alpha2_w, fc_alpha2_b,
           mlp_fc_w, mlp_fc_b, mlp_proj_w, mlp_proj_b):
    f32 = np.float32
    x = np.asarray(x, f32)
    encoder_features = np.asarray(encoder_features, f32)

    # ---- fold LN gains/biases into consumer weights ----
    g1 = np.asarray(ln1_g, f32); b1 = np.asarray(ln1_b, f32)
    g2 = np.asarray(ln2_g, f32); b2 = np.asarray(ln2_b, f32)

    def fold(w, b, g, lb):
        w = np.asarray(w, f32); b = np.asarray(b, f32)
        return (w * g[:, None]).astype(f32), (lb @ w + b).astype(f32)

    w_qkv, b_qkv = fold(c_attn_w, c_attn_b, g1, b1)
    w_fcq, b_fcq = fold(fcq_w, fcq_b, g1, b1)
    w_fck, b_fck = fold(fck_w, fck_b, g1, b1)
    w_fcv, b_fcv = fold(fcv_w, fcv_b, g1, b1)
    w_mfc, b_mfc = fold(mlp_fc_w, mlp_fc_b, g2, b2)
    w_ma = np.asarray(mem_alpha_w, f32)
    w_a1 = np.asarray(fc_alpha1_w, f32)
    w_a2 = np.asarray(fc_alpha2_w, f32)
    w_pj = np.asarray(mlp_proj_w, f32)

    # ---- memory slots (batch independent) ----
    mem = (np.asarray(memory_features, f32)[0] @ np.asarray(mem_attn_w, f32)
           + np.asarray(mem_attn_b, f32))          # [M, 2E]
    mk = mem[:, :E].reshape(M, H, D)
    mv = mem[:, E:].reshape(M, H, D)
    mkT = np.zeros((128, KC, M), f32)
    mvA = np.zeros((M, H * 65), f32)
    for h in range(H):
        c, off = divmod(h, 2)
        mkT[off * 64:(off + 1) * 64, c, :] = mk[:, h, :].T
        mvA[:, h * 65:h * 65 + 64] = mv[:, h, :]
        mvA[:, h * 65 + 64] = 1.0

    bcols = _pack_bias_cols([
        (BC_Q, b_qkv[0:E]), (BC_K, b_qkv[E:2 * E]),
        (BC_PROJ, np.asarray(attn_proj_b, f32)),
        (BC_MA, np.asarray(mem_alpha_b, f32)),
        (BC_FCQ, b_fcq), (BC_FCK, b_fck),
        (BC_EP, np.asarray(enc_proj_b, f32)),
        (BC_A1, np.asarray(fc_alpha1_b, f32)),
        (BC_A2, np.asarray(fc_alpha2_b, f32)),
        (BC_FC, b_mfc), (BC_PJ, np.asarray(mlp_proj_b, f32)),
    ])

    slices = [
        w_qkv[:, 0:E], w_qkv[:, E:2 * E], w_qkv[:, 2 * E:3 * E],
        np.asarray(attn_proj_w, f32),
        w_ma[0:E, :], w_ma[E:2 * E, :],
        w_fcq, w_fck, w_fcv, np.asarray(enc_proj_w, f32),
        w_a1[0:E, :], w_a1[E:2 * E, :], w_a2[0:E, :], w_a2[E:2 * E, :],
        w_mfc[:, 0:E], w_mfc[:, E:2 * E], w_mfc[:, 2 * E:3 * E],
        w_mfc[:, 3 * E:4 * E],
        w_pj[0:E, :], w_pj[E:2 * E, :], w_pj[2 * E:3 * E, :],
        w_pj[3 * E:4 * E, :],
    ]
    wpk = np.stack([_pack_w(s) for s in slices])   # [NW, 128, 6, 768]

    keep = (~np.asarray(mask_encoder, bool)[:, 0, 0, :]).astype(f32)  # [B, SE]

    common = dict(
        wpk=wpk, mkT=mkT, mvA=mvA, bcols=bcols,
        bv_self=np.ascontiguousarray(b_qkv[2 * E:3 * E]).astype(np.float16),
        bv_enc=b_fcv.astype(np.float16),
    )

    in_maps = []
    for core in range(8):
        b, half = divmod(core, 2)
        xTb = x[b].T                                            # [E, S]
        x_pct = np.ascontiguousarray(
            xTb.reshape(KC, 128, S).transpose(1, 0, 2))         # [128,6,S]
        xq_pct = np.ascontiguousarray(
            x_pct[:, :, half * Q:(half + 1) * Q])
        encT = encoder_features[b].transpose(0, 2, 1)           # [2, E, SE]
        enc_pct = np.ascontiguousarray(
            encT.reshape(2, KC, 128, SE).transpose(0, 2, 1, 3))
        m = dict(common)
        m["x_pct"] = x_pct
        m["xq_pct"] = xq_pct
        m["enc_pct"] = enc_pct
        m["maskmul"] = np.ascontiguousarray(keep[b].reshape(NKT, 128).T)
        in_maps.append(m)

    nc = _get_nc()
    res = run_bass_kernel_spmd(nc, in_maps, core_ids=list(range(8)))

    global _LAST_IN_MAPS
    _LAST_IN_MAPS = in_maps

    y = np.empty((B, S, E), f32)
    for core in range(8):
        b, half = divmod(core, 2)
        op = res.results[core]["outP"]                          # [128, 6, Q]
        y[b, half * Q:(half + 1) * Q, :] = (
            op.transpose(1, 0, 2).reshape(E, Q).T)
    return y


_LAST_IN_MAPS = None


def profile_exec_ns(n_hot=40, n_cold=10):
    """Estimate per-invocation device time by timing pipelined repeats of the
    jitted 8-core executable with device-resident inputs."""
    import time
    import jax
    from jax.sharding import Mesh, PartitionSpec
    from jax.experimental.shard_map import shard_map
    import concourse.mybir as mybir_
    from concourse import bass2jax

    if _LAST_IN_MAPS is None:
        return None
    nc = _get_nc()
    in_maps = _LAST_IN_MAPS
    n_cores = 8
    bass2jax.install_neuronx_cc_hook()

    in_names, out_names, out_avals, zero_outs = [], [], [], []
    partition_name = nc.partition_id_tensor.name if nc.partition_id_tensor else None
    for alloc in nc.m.functions[0].allocations:
        if not isinstance(alloc, mybir_.MemoryLocationSet):
            continue
        name = alloc.memorylocations[0].name
        if alloc.kind == "ExternalInput":
            if name != partition_name:
                in_names.append(name)
        elif alloc.kind == "ExternalOutput":
            out_avals.append(jax.core.ShapedArray(
                tuple(alloc.tensor_shape), mybir_.dt.np(alloc.dtype)))
            zero_outs.append(np.zeros(tuple(alloc.tensor_shape),
                                      mybir_.dt.np(alloc.dtype)))
            out_names.append(name)
    n_params = len(in_names)
    n_outs = len(out_avals)
    all_in_names = in_names + out_names + ([partition_name] if partition_name else [])
    donate = tuple(range(n_params, n_params + n_outs))

    def _body(*args):
        operands = list(args)
        if partition_name is not None:
            operands.append(bass2jax.partition_id_tensor())
        return tuple(bass2jax._bass_exec_p.bind(
            *operands, out_avals=tuple(out_avals), in_names=tuple(all_in_names),
            out_names=tuple(out_names), lowering_input_output_aliases=(),
            sim_require_finite=True, sim_require_nnan=True, nc=nc))

    devices = jax.devices()[:n_cores]
    mesh = Mesh(np.asarray(devices), ("core",))
    fn = jax.jit(shard_map(_body, mesh=mesh,
                           in_specs=(PartitionSpec("core"),) * (n_params + n_outs),
                           out_specs=(PartitionSpec("core"),) * n_outs,
                           check_rep=False),
                 donate_argnums=donate, keep_unused=True)
    sh = jax.sharding.NamedSharding(mesh, PartitionSpec("core"))
    concat_in = [jax.device_put(
        np.concatenate([np.asarray(in_maps[c][nm]) for c in range(n_cores)], 0), sh)
        for nm in in_names]

    def zeros():
        return [jax.device_put(
            np.zeros((n_cores * z.shape[0], *z.shape[1:]), z.dtype), sh)
            for z in zero_outs]

    def run(n):
        o = tuple(zeros())
        o = fn(*concat_in, *o)
        jax.block_until_ready(o)
        t0 = time.perf_counter()
        for _ in range(n):
            o = fn(*concat_in, *o)
        jax.block_until_ready(o)
        return time.perf_counter() - t0

    tc = run(n_cold)
    th = run(n_hot)
    per = (th - tc) / (n_hot - n_cold)
    print(f"pipelined wall: {n_cold} calls {tc*1e3:.2f} ms, "
          f"{n_hot} calls {th*1e3:.2f} ms -> per-call {per*1e6:.0f} us")
    return int(per * 1e9)


# revision 34
# speedup vs baseline: 1.1562x; 1.1562x over previous
"""Trainium2 Bass kernel for nn_Block_23338852286694 (dense transformer block).

Sharding: 8 cores = 4 batches x 2 query-halves. Each core computes the full
block for its 512 query tokens (K/V work over the full 1024-token sequence is
duplicated across the pair of cores sharing a batch; no collectives).

On-chip dataflow is feature-major (activations stored transposed, [E, T]).
All weights are host-prepacked into [128, 6, 768] slices so every weight load
is a single 128-descriptor DMA (18KB/partition contiguous). Attention outputs
stay in SBUF via partition-offset engine writes (no DRAM staging); softmax
denominators are batched per attention pass and broadcast with a
selection-matrix matmul (no DMA bounces). All matmuls run in float32r.
"""
import numpy as np

import concourse.bass as bass
import concourse.bacc as bacc
import concourse.mybir as mybir
import concourse.tile as tile
from concourse.bass_utils import run_bass_kernel_spmd

F32 = mybir.dt.float32
F16 = mybir.dt.float16
F32R = mybir.dt.float32r
AF = mybir.ActivationFunctionType
ALU = mybir.AluOpType

B, S, SE = 4, 1024, 1024
E, H, M, D = 768, 12, 100, 64
KC = E // 128             # 6 feature chunks
Q = S // 2                # 512 query tokens per core
EPS = 1e-5
NKT = S // 128            # 8 key tiles
RSQ2 = float(1.0 / np.sqrt(2.0))

# packed per-partition bias column offsets
BC_Q, BC_K, BC_PROJ, BC_MA, BC_FCQ, BC_FCK, BC_EP, BC_A1, BC_A2, BC_FC, BC_PJ = (
    0, 6, 12, 18, 24, 30, 36, 42, 48, 54, 78)
NB = 84

# weight slice order in the packed wpk tensor [NW, 128, KC, 768]
(W_QKQ, W_QKK, W_VS, W_PROJ, W_MA0, W_MA1, W_Q, W_K, W_V, W_EP,
 W_A10, W_A11, W_A20, W_A21, W_FC0, W_FC1, W_FC2, W_FC3,
 W_PJ00, W_PJ01, W_PJ10, W_PJ11) = range(22)
NW = 22


def _row_bcast_dram(ap, parts):
    """DRAM row [N] -> AP readable as [parts, N] (partition-broadcast)."""
    return bass.AP(tensor=ap.tensor, offset=ap.offset,
                   ap=[[0, parts], list(ap.ap[-1])])


def build_program():
    nc = bacc.Bacc(trn_type="TRN2")

    x_pct = nc.dram_tensor("x_pct", [128, KC, S], F32R, kind="ExternalInput")
    xq_pct = nc.dram_tensor("xq_pct", [128, KC, Q], F32R, kind="ExternalInput")
    enc_pct = nc.dram_tensor("enc_pct", [2, 128, KC, SE], F32R,
                             kind="ExternalInput")
    wpk = nc.dram_tensor("wpk", [NW, 128, KC, 768], F32R, kind="ExternalInput")
    maskmul = nc.dram_tensor("maskmul", [128, NKT], F32, kind="ExternalInput")
    mkT = nc.dram_tensor("mkT", [128, KC, M], F32R, kind="ExternalInput")
    mvA = nc.dram_tensor("mvA", [M, H * 65], F32R, kind="ExternalInput")
    bcols = nc.dram_tensor("bcols", [128, NB], F32, kind="ExternalInput")
    bv_self = nc.dram_tensor("bv_self", [E], F16, kind="ExternalInput")
    bv_enc = nc.dram_tensor("bv_enc", [E], F16, kind="ExternalInput")
    outP = nc.dram_tensor("outP", [128, KC, Q], F32, kind="ExternalOutput")

    with tile.TileContext(nc) as tc:
        with nc.allow_low_precision(reason="f32r tiles store full fp32 bits"):
            _emit(nc, tc, x_pct, xq_pct, enc_pct, wpk, maskmul, mkT, mvA,
                  bcols, bv_self, bv_enc, outP)
    nc.compile()
    return nc


def _emit(nc, tc, x_pct, xq_pct, enc_pct, wpk, maskmul, mkT, mvA, bcols,
          bv_self, bv_enc, outP):
    from contextlib import ExitStack
    ctx = ExitStack()
    with ctx:
        consts = ctx.enter_context(tc.tile_pool(name="consts", bufs=1))
        wp = ctx.enter_context(tc.tile_pool(name="wp", bufs=2))
        big24 = ctx.enter_context(tc.tile_pool(name="big24", bufs=1))
        a24 = ctx.enter_context(tc.tile_pool(name="a24", bufs=1))
        q12 = ctx.enter_context(tc.tile_pool(name="q12", bufs=1))
        h12 = ctx.enter_context(tc.tile_pool(name="h12", bufs=1))
        r12 = ctx.enter_context(tc.tile_pool(name="r12", bufs=1))
        m12 = ctx.enter_context(tc.tile_pool(name="m12", bufs=1))
        sm12 = ctx.enter_context(tc.tile_pool(name="sm12", bufs=1))
        vsbp = ctx.enter_context(tc.tile_pool(name="vsbp", bufs=1))
        kchp = ctx.enter_context(tc.tile_pool(name="kchp", bufs=2))
        bcp = ctx.enter_context(tc.tile_pool(name="bcp", bufs=2))
        tmp = ctx.enter_context(tc.tile_pool(name="tmp", bufs=2))
        ptp = ctx.enter_context(tc.tile_pool(name="ptp", bufs=2))
        rcpp = ctx.enter_context(tc.tile_pool(name="rcpp", bufs=2))
        plin = ctx.enter_context(tc.tile_pool(name="plin", bufs=2, space="PSUM"))
        psc = ctx.enter_context(tc.tile_pool(name="psc", bufs=2, space="PSUM"))
        pav = ctx.enter_context(tc.tile_pool(name="pav", bufs=2, space="PSUM"))

        # ---- constants ----
        ones_mm = consts.tile([128, 1], F32R)
        onesf = consts.tile([128, 12], F32)
        nc.vector.memset(onesf, 1.0)
        nc.vector.tensor_copy(out=ones_mm, in_=onesf[:, 0:1])
        ones_f32 = consts.tile([128, 128], F32)
        nc.vector.memset(ones_f32, 1.0)
        ones_bc = consts.tile([128, 128], F32R)  # broadcast rows of 1.0
        nc.vector.tensor_copy(out=ones_bc, in_=ones_f32)
        oneE_bc = consts.tile([128, 128], F32R)  # broadcast rows of 1/E
        nc.vector.tensor_scalar(out=oneE_bc, in0=ones_f32,
                                scalar1=float(1.0 / E), scalar2=None,
                                op0=ALU.mult)
        bc = consts.tile([128, NB], F32)
        nc.sync.dma_start(out=bc, in_=bcols[:, :])
        mm_sb = consts.tile([128, NKT], F32)
        nc.sync.dma_start(out=mm_sb, in_=maskmul[:, :])
        mk_sb = consts.tile([128, KC, M], F32R)
        nc.sync.dma_start(out=mk_sb, in_=mkT[:, :, :])
        mv_sb = consts.tile([M, H * 65], F32R)
        nc.sync.dma_start(out=mv_sb, in_=mvA[:, :])
        bvb_s = consts.tile([128, E], F16)
        nc.sync.dma_start(out=bvb_s, in_=_row_bcast_dram(bv_self[:], 128))
        bvb_e = consts.tile([128, E], F16)
        nc.sync.dma_start(out=bvb_e, in_=_row_bcast_dram(bv_enc[:], 128))
        eps_t = consts.tile([128, 1], F32)
        nc.vector.memset(eps_t, EPS)

        def load_w(idx, name="wt"):
            t = wp.tile([128, KC, 768], F32R, tag="w18", name=name)
            nc.sync.dma_start(out=t, in_=wpk[idx, :, :, :])
            return t

        def layernorm(chunks, T, out_chunks=None):
            """Feature-major layernorm. chunks: KC APs [128, T] f32r.
            out_chunks None -> in place."""
            nt = T // 512
            inplace = out_chunks is None
            srows = []
            for t in range(nt):
                ps_s = plin.tile([128, 512], F32, tag="lin", name="ps_s")
                ps_q = plin.tile([128, 512], F32, tag="lin", name="ps_q")
                sl = slice(t * 512, (t + 1) * 512)
                for c in range(KC):
                    src = chunks[c][:, sl]
                    sq = tmp.tile([128, 512], F32R, tag="t2", name="sq")
                    eng = nc.gpsimd if c % 2 else nc.vector
                    eng.tensor_tensor(out=sq, in0=src.bitcast(F32),
                                      in1=src.bitcast(F32), op=ALU.mult)
                    nc.tensor.matmul(ps_s[0:1, :], ones_mm, src,
                                     start=(c == 0), stop=(c == KC - 1))
                    nc.tensor.matmul(ps_q[0:1, :], ones_mm, sq,
                                     start=(c == 0), stop=(c == KC - 1))
                sr = tmp.tile([128, 512], F32R, tag="t2", name="sr")
                nc.scalar.activation(out=sr[0:1, :], in_=ps_s[0:1, :],
                                     func=AF.Identity, scale=1.0)
                nc.scalar.activation(out=sr[32:33, :], in_=ps_q[0:1, :],
                                     func=AF.Identity, scale=1.0)
                srows.append(sr)
            mu_b = psc.tile([128, 1024], F32, tag="sc", name="mub")
            e2_b = psc.tile([128, 1024], F32, tag="sc", name="e2b")
            for t in range(nt):
                sl = slice(t * 512, (t + 1) * 512)
                nc.tensor.matmul(mu_b[:, sl], oneE_bc[0:1, :],
                                 srows[t][0:1, :], start=True, stop=True)
                nc.tensor.matmul(e2_b[:, sl], oneE_bc[32:33, :],
                                 srows[t][32:33, :], start=True, stop=True)
            rs_sb = bcp.tile([128, 1024], F16, tag="bc8", name="rs")
            murs_sb = bcp.tile([128, 1024], F16, tag="bc8", name="murs")
            for t in range(nt):
                sl = slice(t * 512, (t + 1) * 512)
                mu_s = tmp.tile([128, 512], F32, tag="t2", name="mu_s")
                nc.vector.tensor_copy(out=mu_s, in_=mu_b[:, sl])
                msq = tmp.tile([128, 512], F32, tag="t2", name="msq")
                nc.vector.tensor_tensor(out=msq, in0=mu_s, in1=mu_s,
                                        op=ALU.mult)
                nc.vector.tensor_tensor(out=e2_b[:, sl], in0=e2_b[:, sl],
                                        in1=msq, op=ALU.subtract)
                sd = tmp.tile([128, 512], F32, tag="t2", name="sd")
                nc.scalar.activation(out=sd, in_=e2_b[:, sl], func=AF.Sqrt,
                                     bias=eps_t[:, :], scale=1.0)
                nc.vector.reciprocal(out=rs_sb[:, sl], in_=sd)
                nc.vector.tensor_tensor(out=murs_sb[:, sl], in0=mu_s,
                                        in1=rs_sb[:, sl], op=ALU.mult)
            for c in range(KC):
                for t in range(nt):
                    sl = slice(t * 512, (t + 1) * 512)
                    src = chunks[c][:, sl]
                    o = src if inplace else out_chunks[c][:, sl]
                    eng = nc.gpsimd if c % 2 else nc.vector
                    eng.tensor_tensor(out=o, in0=src.bitcast(F32),
                                      in1=rs_sb[:, sl], op=ALU.mult)
                    nc.vector.tensor_tensor(out=o, in0=o.bitcast(F32),
                                            in1=murs_sb[:, sl],
                                            op=ALU.subtract)

        def lin_psum(ft):
            """Alternate linear accumulators across both PSUM pools for a
            4-deep drain pipeline (only outside attention phases)."""
            if ft % 2:
                return psc.tile([128, 1024], F32, tag="sc", name="lps")[:, 0:512]
            return plin.tile([128, 512], F32, tag="lin", name="lp")[:, :]

        def linear_ft(ft, terms, bias_col, act, out_ap, alt=True):
            """terms: list of (wtile, kc_in_tile, rhs [128, Q] AP)."""
            ps = lin_psum(ft) if alt else plin.tile(
                [128, 512], F32, tag="lin", name="lp")[:, :]
            n = len(terms)
            for i, (wt, kci, rhs) in enumerate(terms):
                nc.tensor.matmul(ps, wt[:, kci, ft * 128:(ft + 1) * 128],
                                 rhs, start=(i == 0), stop=(i == n - 1))
            nc.scalar.activation(
                out=out_ap, in_=ps, func=act,
                bias=bc[:, bias_col + ft:bias_col + ft + 1], scale=1.0)

        def make_v(v_tile, src_fn, wv, bias_b, masked):
            """V production into v_tile [128, NKT, H*65] (token-major)."""
            for tt in range(NKT):
                for h0 in (0, 6):
                    c0 = h0 * 64
                    ps = plin.tile([128, 512], F32, tag="lin", name="vps")
                    for kc in range(KC):
                        nc.tensor.matmul(
                            ps[:, 0:384],
                            src_fn(kc)[:, tt * 128:(tt + 1) * 128],
                            wv[:, kc, c0:c0 + 384],
                            start=(kc == 0), stop=(kc == KC - 1))
                    vrow = v_tile[:, tt, :].rearrange("p (h c) -> p h c", c=65)
                    nc.vector.tensor_tensor(
                        out=vrow[:, h0:h0 + 6, 0:64],
                        in0=ps[:, 0:384].rearrange("p (h c) -> p h c", c=64),
                        in1=bias_b[:, c0:c0 + 384].rearrange(
                            "p (h c) -> p h c", c=64),
                        op=ALU.add)
                if masked:
                    nc.vector.tensor_scalar(
                        out=v_tile[:, tt, :],
                        in0=v_tile[:, tt, :].bitcast(F32),
                        scalar1=mm_sb[:, tt:tt + 1], scalar2=None,
                        op0=ALU.mult)

        def init_ones_cols(v_tile):
            for tt in range(NKT):
                vrow = v_tile[:, tt, :].rearrange("p (h c) -> p h c", c=65)
                nc.vector.tensor_copy(
                    out=vrow[:, :, 64:65],
                    in_=onesf[:, :].rearrange("p (h o) -> p h o", o=1))

        def attention(kch_fn, v_tile, q_tile, out24, scale, with_mem):
            """q_tile [128, KC, Q]; raw (unnormalized) head outputs into
            out24[:, c, :] rows off..off+64 (cols 6+c for memory); per-chunk
            raw denominators staged at rows 0/32 of a small tile, broadcast
            with K=1 ones-matmuls, then one reciprocal+multiply per chunk."""
            for c in range(KC):
                kch = kch_fn(c)
                den2 = rcpp.tile([33, 512], F32R, tag="den", name="den2")
                den2m = None
                if with_mem:
                    den2m = rcpp.tile([33, 512], F32R, tag="den", name="den2m")
                for hh in range(2):
                    h = 2 * c + hh
                    off = hh * 64
                    dr = 32 * hh
                    av = pav.tile([65, 512], F32, tag="av", name="av")
                    for half in range(NKT // 2):
                        sc = psc.tile([128, 1024], F32, tag="sc", name="sc")
                        for j in range(2):
                            kt = half * 2 + j
                            nc.tensor.matmul(
                                sc[:, j * 512:(j + 1) * 512],
                                kch[off:off + 64, kt * 128:(kt + 1) * 128],
                                q_tile[off:off + 64, c, :],
                                start=True, stop=True)
                        pt = ptp.tile([128, 1024], F32R, tag="pt", name="pt")
                        nc.scalar.activation(out=pt, in_=sc[:, :], func=AF.Exp,
                                             scale=scale)
                        for j in range(2):
                            kt = half * 2 + j
                            nc.tensor.matmul(
                                av[:, :], v_tile[:, kt, h * 65:(h + 1) * 65],
                                pt[:, j * 512:(j + 1) * 512],
                                start=(kt == 0), stop=(kt == NKT - 1))
                    nc.vector.tensor_copy(out=den2[dr:dr + 1, :],
                                          in_=av[64:65, :])
                    if hh == 0:
                        nc.scalar.activation(out=out24[off:off + 64, c, :],
                                             in_=av[0:64, :], func=AF.Identity,
                                             scale=1.0)
                    else:
                        nc.vector.tensor_copy(out=out24[off:off + 64, c, :],
                                              in_=av[0:64, :])
                    if with_mem:
                        scm = plin.tile([128, 512], F32, tag="lin", name="scm")
                        nc.tensor.matmul(scm[0:M, 0:512],
                                         mk_sb[off:off + 64, c, :],
                                         q_tile[off:off + 64, c, :],
                                         start=True, stop=True)
                        pmt = ptp.tile([128, 1024], F32R, tag="pt", name="pmt")
                        nc.scalar.activation(out=pmt[0:M, 0:512],
                                             in_=scm[0:M, :],
                                             func=AF.Exp, scale=1.0)
                        av1 = pav.tile([65, 512], F32, tag="av", name="av1")
                        nc.tensor.matmul(av1[:, :],
                                         mv_sb[:, h * 65:(h + 1) * 65],
                                         pmt[0:M, 0:512], start=True,
                                         stop=True)
                        nc.scalar.activation(out=den2m[dr:dr + 1, :],
                                             in_=av1[64:65, :],
                                             func=AF.Identity, scale=1.0)
                        if hh == 0:
                            nc.scalar.activation(
                                out=out24[off:off + 64, KC + c, :],
                                in_=av1[0:64, :], func=AF.Identity, scale=1.0)
                        else:
                            nc.vector.tensor_copy(
                                out=out24[off:off + 64, KC + c, :],
                                in_=av1[0:64, :])
                blocks = [(0, den2)] + ([(KC, den2m)] if with_mem else [])
                for blk, dt in blocks:
                    bpa = plin.tile([128, 512], F32, tag="lin", name="bpa")
                    bpb = plin.tile([128, 512], F32, tag="lin", name="bpb")
                    nc.tensor.matmul(bpa[0:64, :], ones_bc[0:1, 0:64],
                                     dt[0:1, :], start=True, stop=True)
                    nc.tensor.matmul(bpb[0:64, :], ones_bc[32:33, 0:64],
                                     dt[32:33, :], start=True, stop=True)
                    nc.vector.reciprocal(out=bpa[0:64, :], in_=bpa[0:64, :])
                    nc.vector.reciprocal(out=bpb[0:64, :], in_=bpb[0:64, :])
                    nc.vector.tensor_tensor(
                        out=out24[0:64, blk + c, :],
                        in0=out24[0:64, blk + c, :].bitcast(F32),
                        in1=bpa[0:64, :], op=ALU.mult)
                    nc.vector.tensor_tensor(
                        out=out24[64:128, blk + c, :],
                        in0=out24[64:128, blk + c, :].bitcast(F32),
                        in1=bpb[0:64, :], op=ALU.mult)

        # ---- persistent tiles ----
        v_sb = vsbp.tile([128, NKT, H * 65], F32R, tag="vsb")
        init_ones_cols(v_sb)

        # ======== phase A: load x, LN in place ========
        xh = big24.tile([128, KC, S], F32R, tag="big24", name="xh")
        for c in range(KC):
            nc.sync.dma_start(out=xh[:, c, :], in_=x_pct[:, c, :])
        xqh = h12.tile([128, KC, Q], F32R, tag="h12", name="xqh")
        nc.sync.dma_start(out=xqh, in_=xq_pct[:, :, :])
        layernorm([xh[:, c, :] for c in range(KC)], S)
        layernorm([xqh[:, c, :] for c in range(KC)], Q)

        # ======== phase B: v, q, k, self+memory attention ========
        w_vs = load_w(W_VS, "wvs")
        make_v(v_sb, lambda kc: xh[:, kc, :], w_vs, bvb_s, masked=False)
        w_qq = load_w(W_QKQ, "wqq")
        qT = q12.tile([128, KC, Q], F32R, tag="q12", name="qT")
        for ft in range(KC):
            linear_ft(ft, [(w_qq, kc, xqh[:, kc, :]) for kc in range(KC)],
                      BC_Q, AF.Identity, qT[:, ft, :])
        w_qk = load_w(W_QKK, "wqk")

        def self_kchunk(c):
            kt = kchp.tile([128, S], F32R, tag="kch", name="kchs")
            for t in range(2):
                sl = slice(t * 512, (t + 1) * 512)
                ps = plin.tile([128, 512], F32, tag="lin", name="kps")
                for kc in range(KC):
                    nc.tensor.matmul(ps[:, :],
                                     w_qk[:, kc, c * 128:(c + 1) * 128],
                                     xh[:, kc, sl], start=(kc == 0),
                                     stop=(kc == KC - 1))
                nc.scalar.activation(out=kt[:, sl], in_=ps[:, :],
                                     func=AF.Identity,
                                     bias=bc[:, BC_K + c:BC_K + c + 1],
                                     scale=1.0)
            return kt

        gAB = a24.tile([128, 12, Q], F32R, tag="a24", name="gAB")
        attention(self_kchunk, v_sb, qT, gAB, 1.0, with_mem=True)

        # ======== phase B4: memory gate + attn_proj + residual ========
        w_ma0 = load_w(W_MA0, "wma0")
        w_ma1 = load_w(W_MA1, "wma1")
        aN = q12.tile([128, KC, Q], F32R, tag="q12", name="aN")
        for ft in range(KC):
            ps = lin_psum(ft)
            for i in range(12):
                wt = w_ma0 if i < KC else w_ma1
                nc.tensor.matmul(ps,
                                 wt[:, i % KC, ft * 128:(ft + 1) * 128],
                                 gAB[:, i, :], start=(i == 0), stop=(i == 11))
            al = tmp.tile([128, 512], F32, tag="t2", name="al")
            nc.scalar.activation(out=al, in_=ps, func=AF.Sigmoid,
                                 bias=bc[:, BC_MA + ft:BC_MA + ft + 1],
                                 scale=1.0)
            d = tmp.tile([128, 512], F32, tag="t2", name="d")
            nc.vector.tensor_tensor(out=d, in0=gAB[:, ft, :].bitcast(F32),
                                    in1=gAB[:, KC + ft, :].bitcast(F32),
                                    op=ALU.subtract)
            eng = nc.gpsimd if ft % 2 else nc.vector
            eng.tensor_tensor(out=d, in0=al, in1=d, op=ALU.mult)
            nc.vector.tensor_tensor(out=aN[:, ft, :],
                                    in0=gAB[:, KC + ft, :].bitcast(F32),
                                    in1=d, op=ALU.add)
        w_proj = load_w(W_PROJ, "wproj")
        xq_raw = m12.tile([128, KC, Q], F32R, tag="m12", name="xq_raw")
        nc.sync.dma_start(out=xq_raw, in_=xq_pct[:, :, :])
        aT = sm12.tile([128, KC, Q], F32R, tag="sm12", name="aT")
        for ft in range(KC):
            ps = lin_psum(ft)
            for kc in range(KC):
                nc.tensor.matmul(ps,
                                 w_proj[:, kc, ft * 128:(ft + 1) * 128],
                                 aN[:, kc, :], start=(kc == 0),
                                 stop=(kc == KC - 1))
            nc.vector.scalar_tensor_tensor(
                out=aT[:, ft, :], in0=ps,
                scalar=bc[:, BC_PROJ + ft:BC_PROJ + ft + 1],
                in1=xq_raw[:, ft, :].bitcast(F32), op0=ALU.add, op1=ALU.add)

        # ======== phase C: hahat + qe ========
        hah = h12.tile([128, KC, Q], F32R, tag="h12", name="hah")
        layernorm([aT[:, c, :] for c in range(KC)], Q,
                  [hah[:, c, :] for c in range(KC)])
        w_fcq = load_w(W_Q, "wfcq")
        # encoder 0 load + LN (overlaps C/D0 start)
        eh0 = big24.tile([128, KC, SE], F32R, tag="big24", name="eh0")
        for c in range(KC):
            nc.sync.dma_start(out=eh0[:, c, :], in_=enc_pct[0, :, c, :])
        layernorm([eh0[:, c, :] for c in range(KC)], SE)
        qeT = q12.tile([128, KC, Q], F32R, tag="q12", name="qeT")
        for ft in range(KC):
            linear_ft(ft, [(w_fcq, kc, hah[:, kc, :]) for kc in range(KC)],
                      BC_FCQ, AF.Identity, qeT[:, ft, :])

        # ======== phase D: two cross-attentions ========
        e1p = r12.tile([128, KC, Q], F32, tag="r12", name="e1p")
        asum = m12.tile([128, KC, Q], F32R, tag="m12", name="asum")

        def enc_attention(eh, out24):
            w_fcv = load_w(W_V, "wfcv")
            make_v(v_sb, lambda kc: eh[:, kc, :], w_fcv, bvb_e, masked=True)
            w_fck = load_w(W_K, "wfck")

            def enc_kchunk(c):
                kt = kchp.tile([128, S], F32R, tag="kch", name="kche")
                for t in range(2):
                    sl = slice(t * 512, (t + 1) * 512)
                    ps = plin.tile([128, 512], F32, tag="lin", name="keps")
                    for kc in range(KC):
                        nc.tensor.matmul(ps[:, :],
                                         w_fck[:, kc, c * 128:(c + 1) * 128],
                                         eh[:, kc, sl], start=(kc == 0),
                                         stop=(kc == KC - 1))
                    nc.scalar.activation(out=kt[:, sl], in_=ps[:, :],
                                         func=AF.Identity,
                                         bias=bc[:, BC_FCK + c:BC_FCK + c + 1],
                                         scale=1.0)
                return kt

            attention(enc_kchunk, v_sb, qeT, out24, 0.125, with_mem=False)

        def enc_proj(ee):
            w_ep = load_w(W_EP, "wep")
            for ft in range(KC):
                linear_ft(ft, [(w_ep, kc, ee[:, kc, :]) for kc in range(KC)],
                          BC_EP, AF.Identity, ee[:, KC + ft, :])

        def enc_gates(e, ee, wa0, wa1):
            bcol0 = BC_A1 if e == 0 else BC_A2
            for ft in range(KC):
                ps = lin_psum(ft)
                for i in range(12):
                    wt = wa0 if i < KC else wa1
                    rhs = aT[:, i, :] if i < KC else ee[:, i, :]
                    nc.tensor.matmul(ps,
                                     wt[:, i % KC, ft * 128:(ft + 1) * 128],
                                     rhs, start=(i == 0), stop=(i == 11))
                al = tmp.tile([128, 512], F32, tag="t2", name="alE")
                nc.scalar.activation(out=al, in_=ps, func=AF.Sigmoid,
                                     bias=bc[:, bcol0 + ft:bcol0 + ft + 1],
                                     scale=1.0)
                d = tmp.tile([128, 512], F32, tag="t2", name="dE")
                nc.vector.tensor_tensor(out=d, in0=aT[:, ft, :].bitcast(F32),
                                        in1=ee[:, KC + ft, :].bitcast(F32),
                                        op=ALU.subtract)
                eng = nc.gpsimd if ft % 2 else nc.vector
                eng.tensor_tensor(out=d, in0=al, in1=d, op=ALU.mult)
                if e == 0:
                    nc.vector.tensor_tensor(out=e1p[:, ft, :],
                                            in0=ee[:, KC + ft, :].bitcast(F32),
                                            in1=d, op=ALU.add)
                else:
                    nc.vector.tensor_tensor(out=d,
                                            in0=ee[:, KC + ft, :].bitcast(F32),
                                            in1=d, op=ALU.add)
                    nc.vector.tensor_tensor(out=asum[:, ft, :],
                                            in0=e1p[:, ft, :], in1=d,
                                            op=ALU.add)

        ee0 = a24.tile([128, 12, Q], F32R, tag="a24", name="ee0")
        enc_attention(eh0, ee0)
        enc_proj(ee0)
        wa0 = load_w(W_A10, "wa0")
        wa1 = load_w(W_A11, "wa1")
        # encoder 1 load + LN (overlaps e0 ep/gates)
        eh1 = big24.tile([128, KC, SE], F32R, tag="big24", name="eh1")
        for c in range(KC):
            nc.sync.dma_start(out=eh1[:, c, :], in_=enc_pct[1, :, c, :])
        layernorm([eh1[:, c, :] for c in range(KC)], SE)
        enc_gates(0, ee0, wa0, wa1)
        ee1 = a24.tile([128, 12, Q], F32R, tag="a24", name="ee1")
        enc_attention(eh1, ee1)
        enc_proj(ee1)
        wa0b = load_w(W_A20, "wa0b")
        wa1b = load_w(W_A21, "wa1b")
        enc_gates(1, ee1, wa0b, wa1b)

        # ======== phase E: MLP + final residual ========
        hm2 = h12.tile([128, KC, Q], F32R, tag="h12", name="hm2")
        layernorm([asum[:, c, :] for c in range(KC)], Q,
                  [hm2[:, c, :] for c in range(KC)])
        mstage = sm12.tile([128, KC, Q], F32, tag="sm12", name="mstage")
        for mh in range(2):
            wfa = load_w(W_FC0 + 2 * mh, "wfa")
            wfb = load_w(W_FC1 + 2 * mh, "wfb")
            mpool = big24 if mh == 0 else a24
            mT = mpool.tile([128, 12, Q], F32R,
                            tag="big24" if mh == 0 else "a24", name="mT")
            for ft in range(12):
                wt = wfa if ft < KC else wfb
                linear_ft(ft % KC,
                          [(wt, kc, hm2[:, kc, :]) for kc in range(KC)],
                          BC_FC + 12 * mh + (0 if ft < KC else KC),
                          AF.Gelu_apprx_tanh, mT[:, ft, :])
            wpa = load_w(W_PJ00 + 2 * mh, "wpa")
            wpb = load_w(W_PJ01 + 2 * mh, "wpb")
            for ft in range(KC):
                ps = lin_psum(ft)
                for i in range(12):
                    wt = wpa if i < KC else wpb
                    nc.tensor.matmul(ps,
                                     wt[:, i % KC, ft * 128:(ft + 1) * 128],
                                     mT[:, i, :], start=(i == 0),
                                     stop=(i == 11))
                if mh == 0:
                    nc.scalar.activation(out=mstage[:, ft, :], in_=ps,
                                         func=AF.Identity,
                                         bias=bc[:, BC_PJ + ft:BC_PJ + ft + 1],
                                         scale=1.0)
                else:
                    t = tmp.tile([128, 512], F32, tag="t2", name="mo")
                    nc.vector.scalar_tensor_tensor(
                        out=t, in0=asum[:, ft, :].bitcast(F32), scalar=RSQ2,
                        in1=ps, op0=ALU.mult, op1=ALU.add)
                    ot = tmp.tile([128, 512], F32, tag="t2", name="ot")
                    nc.vector.tensor_tensor(out=ot, in0=t,
                                            in1=mstage[:, ft, :], op=ALU.add)
                    nc.sync.dma_start(out=outP[:, ft, :], in_=ot)


_NC_CACHE = None


def _get_nc():
    global _NC_CACHE
    if _NC_CACHE is None:
        _NC_CACHE = build_program()
    return _NC_CACHE


def _pack_bias_cols(seg_biases):
    bcols = np.zeros((128, NB), np.float32)
    for col0, b in seg_biases:
        nf = b.shape[0] // 128
        bcols[:, col0:col0 + nf] = b.reshape(nf, 128).T
    return bcols


def _pack_w(blk):
    """[768, F] -> [128, 6, F] per-partition contiguous."""
    return np.ascontiguousarray(
        blk.reshape(KC, 128, blk.shape[1]).transpose(1, 0, 2))


def kernel(x, encoder_features, mask_encoder, ln1_g, ln1_b, ln2_g, ln2_b,
           c_attn_w, c_attn_b, attn_proj_w, attn_proj_b,
           memory_features, mem_attn_w, mem_attn_b, mem_alpha_w, mem_alpha_b,
           fcq_w, fcq_b, fck_w, fck_b, fcv_w, fcv_b, enc_proj_w, enc_proj_b,
           fc_alpha1_w, fc_alpha1_b, fc_alpha2_w, fc_alpha2_b,
           mlp_fc_w, mlp_fc_b, mlp_proj_w, mlp_proj_b):
    f32 = np.float32
    x = np.asarray(x, f32)
    encoder_features = np.asarray(encoder_features, f32)

    # ---- fold LN gains/biases into consumer weights ----
    g1 = np.asarray(ln1_g, f32); b1 = np.asarray(ln1_b, f32)
    g2 = np.asarray(ln2_g, f32); b2 = np.asarray(ln2_b, f32)

    def fold(w, b, g, lb):
        w = np.asarray(w, f32); b = np.asarray(b, f32)
        return (w * g[:, None]).astype(f32), (lb @ w + b).astype(f32)

    w_qkv, b_qkv = fold(c_attn_w, c_attn_b, g1, b1)
    w_fcq, b_fcq = fold(fcq_w, fcq_b, g1, b1)
    w_fck, b_fck = fold(fck_w, fck_b, g1, b1)
    w_fcv, b_fcv = fold(fcv_w, fcv_b, g1, b1)
    w_mfc, b_mfc = fold(mlp_fc_w, mlp_fc_b, g2, b2)
    w_ma = np.asarray(mem_alpha_w, f32)
    w_a1 = np.asarray(fc_alpha1_w, f32)
    w_a2 = np.asarray(fc_alpha2_w, f32)
    w_pj = np.asarray(mlp_proj_w, f32)

    # ---- memory slots (batch independent) ----
    mem = (np.asarray(memory_features, f32)[0] @ np.asarray(mem_attn_w, f32)
           + np.asarray(mem_attn_b, f32))          # [M, 2E]
    mk = mem[:, :E].reshape(M, H, D)
    mv = mem[:, E:].reshape(M, H, D)
    mkT = np.zeros((128, KC, M), f32)
    mvA = np.zeros((M, H * 65), f32)
    for h in range(H):
        c, off = divmod(h, 2)
        mkT[off * 64:(off + 1) * 64, c, :] = mk[:, h, :].T
        mvA[:, h * 65:h * 65 + 64] = mv[:, h, :]
        mvA[:, h * 65 + 64] = 1.0

    bcols = _pack_bias_cols([
        (BC_Q, b_qkv[0:E]), (BC_K, b_qkv[E:2 * E]),
        (BC_PROJ, np.asarray(attn_proj_b, f32)),
        (BC_MA, np.asarray(mem_alpha_b, f32)),
        (BC_FCQ, b_fcq), (BC_FCK, b_fck),
        (BC_EP, np.asarray(enc_proj_b, f32)),
        (BC_A1, np.asarray(fc_alpha1_b, f32)),
        (BC_A2, np.asarray(fc_alpha2_b, f32)),
        (BC_FC, b_mfc), (BC_PJ, np.asarray(mlp_proj_b, f32)),
    ])

    slices = [
        w_qkv[:, 0:E], w_qkv[:, E:2 * E], w_qkv[:, 2 * E:3 * E],
        np.asarray(attn_proj_w, f32),
        w_ma[0:E, :], w_ma[E:2 * E, :],
        w_fcq, w_fck, w_fcv, np.asarray(enc_proj_w, f32),
        w_a1[0:E, :], w_a1[E:2 * E, :], w_a2[0:E, :], w_a2[E:2 * E, :],
        w_mfc[:, 0:E], w_mfc[:, E:2 * E], w_mfc[:, 2 * E:3 * E],
        w_mfc[:, 3 * E:4 * E],
        w_pj[0:E, :], w_pj[E:2 * E, :], w_pj[2 * E:3 * E, :],
        w_pj[3 * E:4 * E, :],
    ]
    wpk = np.stack([_pack_w(s) for s in slices])   # [NW, 128, 6, 768]

    keep = (~np.asarray(mask_encoder, bool)[:, 0, 0, :]).astype(f32)  # [B, SE]

    common = dict(
        wpk=wpk, mkT=mkT, mvA=mvA, bcols=bcols,
        bv_self=np.ascontiguousarray(b_qkv[2 * E:3 * E]).astype(np.float16),
        bv_enc=b_fcv.astype(np.float16),
    )

    in_maps = []
    for core in range(8):
        b, half = divmod(core, 2)
        xTb = x[b].T                                            # [E, S]
        x_pct = np.ascontiguousarray(
            xTb.reshape(KC, 128, S).transpose(1, 0, 2))         # [128,6,S]
        xq_pct = np.ascontiguousarray(
            x_pct[:, :, half * Q:(half + 1) * Q])
        encT = encoder_features[b].transpose(0, 2, 1)           # [2, E, SE]
        enc_pct = np.ascontiguousarray(
            encT.reshape(2, KC, 128, SE).transpose(0, 2, 1, 3))
        m = dict(common)
        m["x_pct"] = x_pct
        m["xq_pct"] = xq_pct
        m["enc_pct"] = enc_pct
        m["maskmul"] = np.ascontiguousarray(keep[b].reshape(NKT, 128).T)
        in_maps.append(m)

    nc = _get_nc()
    res = run_bass_kernel_spmd(nc, in_maps, core_ids=list(range(8)))

    global _LAST_IN_MAPS
    _LAST_IN_MAPS = in_maps

    y = np.empty((B, S, E), f32)
    for core in range(8):
        b, half = divmod(core, 2)
        op = res.results[core]["outP"]                          # [128, 6, Q]
        y[b, half * Q:(half + 1) * Q, :] = (
            op.transpose(1, 0, 2).reshape(E, Q).T)
    return y


_LAST_IN_MAPS = None


def profile_exec_ns(n_hot=12, n_cold=2):
    """Estimate per-invocation device time by timing pipelined repeats of the
    jitted 8-core executable with device-resident inputs."""
    import time
    import jax
    from jax.sharding import Mesh, PartitionSpec
    from jax.experimental.shard_map import shard_map
    import concourse.mybir as mybir_
    from concourse import bass2jax

    if _LAST_IN_MAPS is None:
        return None
    nc = _get_nc()
    in_maps = _LAST_IN_MAPS
    n_cores = 8
    bass2jax.install_neuronx_cc_hook()

    in_names, out_names, out_avals, zero_outs = [], [], [], []
    partition_name = nc.partition_id_tensor.name if nc.partition_id_tensor else None
    for alloc in nc.m.functions[0].allocations:
        if not isinstance(alloc, mybir_.MemoryLocationSet):
            continue
        name = alloc.memorylocations[0].name
        if alloc.kind == "ExternalInput":
            if name != partition_name:
                in_names.append(name)
        elif alloc.kind == "ExternalOutput":
            out_avals.append(jax.core.ShapedArray(
                tuple(alloc.tensor_shape), mybir_.dt.np(alloc.dtype)))
            zero_outs.append(np.zeros(tuple(alloc.tensor_shape),
                                      mybir_.dt.np(alloc.dtype)))
            out_names.append(name)
    n_params = len(in_names)
    n_outs = len(out_avals)
    all_in_names = in_names + out_names + ([partition_name] if partition_name else [])
    donate = tuple(range(n_params, n_params + n_outs))

    def _body(*args):
        operands = list(args)
        if partition_name is not None:
            operands.append(bass2jax.partition_id_tensor())
        return tuple(bass2jax._bass_exec_p.bind(
            *operands, out_avals=tuple(out_avals), in_names=tuple(all_in_names),
            out_names=tuple(out_names), lowering_input_output_aliases=(),
            sim_require_finite=True, sim_require_nnan=True, nc=nc))

    devices = jax.devices()[:n_cores]
    mesh = Mesh(np.asarray(devices), ("core",))
    fn = jax.jit(shard_map(_body, mesh=mesh,
                           in_specs=(PartitionSpec("core"),) * (n_params + n_outs),
                           out_specs=(PartitionSpec("core"),) * n_outs,
                           check_rep=False),
                 donate_argnums=donate, keep_unused=True)
    sh = jax.sharding.NamedSharding(mesh, PartitionSpec("core"))
    concat_in = [jax.device_put(
        np.concatenate([np.asarray(in_maps[c][nm]) for c in range(n_cores)], 0), sh)
        for nm in in_names]

    def zeros():
        return [jax.device_put(
            np.zeros((n_cores * z.shape[0], *z.shape[1:]), z.dtype), sh)
            for z in zero_outs]

    def run(n):
        o = tuple(zeros())
        o = fn(*concat_in, *o)
        jax.block_until_ready(o)
        t0 = time.perf_counter()
        for _ in range(n):
            o = fn(*concat_in, *o)
        jax.block_until_ready(o)
        return time.perf_counter() - t0

    tc = run(n_cold)
    th = run(n_hot)
    per = (th - tc) / (n_hot - n_cold)
    print(f"pipelined wall: {n_cold} calls {tc*1e3:.2f} ms, "
          f"{n_hot} calls {th*1e3:.2f} ms -> per-call {per*1e6:.0f} us")
    return int(per * 1e9)
